# revision 5
# baseline (speedup 1.0000x reference)
"""Trainium2 Bass kernel for nn_GateCircuit (14-qubit batched gate circuit).

Math: the reference applies RX(x@W.T[:,i]) then RY(params[i]) on wire i of
|0...0> (a product state stays a product state since each gate hits a distinct
wire), then a CNOT ladder CNOT(i, i+1), then measures <Z_0>.  Qubit 0 is only
ever a CNOT *control*, so its marginal is untouched by the ladder; the
expectation collapses to the single-qubit value

    <Z_0> = cos(x @ W[0]) * cos(params[0])
    out   = sigmoid(<Z_0>)

Sharding: pure data parallel, batch 4096 split 512 per core across 8 cores.

Host prep (cheap scalar/row transforms only; the 4MB matvec stays on-device):
  w' = W[0] / (2*pi)  broadcast to 128 partitions, packed with the single
  runtime constant 0.5*cos(params[0]) into one [128, 257] tensor -> one DMA.

On-device per core (all f32):
  z' = x @ w'                 4x DVE scalar_tensor_tensor with accumulator
                              (z' = z/2pi; x laid out [128, 4, 256] so rows
                              4p..4p+3 are 2KB-contiguous per DMA half)
  k  = f32(i32(z'))           round-to-nearest on HW cast
  y  = z' - k   in [-.5,.5]   cos(z) = cos(2*pi*y)
  s  = Sin(pi*y)              ACT input within [-pi/2,pi/2]
  q  = Square(s)              = sin^2(z/2);  a = c0*(1-2q)
  t  = Tanh(-c0*q + 0.5*c0)   = tanh(a/2); per-partition AP scale+bias
  o  = Copy(0.5*t + 0.5)
Sin/Tanh/Copy share one ACT table set -> a single ACT_TABLE_LOAD, hoisted to
the top of the Scalar stream.  DMAs: x halves on Sync+Vector queues in
parallel, consts on GpSimd, output on Sync.
"""

import math

import numpy as np

_NCORES = 8
_B = 4096
_F = 256
_BS = _B // _NCORES  # 512 samples per core
_NT = _BS // 128     # 4 samples per partition
_INV_TWO_PI = float(1.0 / (2.0 * math.pi))
_TWO_PI = float(2.0 * math.pi)
_PI = float(math.pi)

_CACHE: dict = {}


def _build():
    import concourse.bacc as bacc
    import concourse.mybir as mybir
    import concourse.tile as tile

    f32 = mybir.dt.float32
    i32 = mybir.dt.int32
    Alu = mybir.AluOpType
    Act = mybir.ActivationFunctionType

    nc = bacc.Bacc("TRN2", target_bir_lowering=False, debug=False,
                   num_devices=_NCORES)

    x_d = nc.dram_tensor("x", [_BS, _F], f32, kind="ExternalInput")
    w_d = nc.dram_tensor("w", [128, _F + 2], f32, kind="ExternalInput")
    o_d = nc.dram_tensor("o", [_BS], f32, kind="ExternalOutput")

    with tile.TileContext(nc) as tc:
        with (
            tc.tile_pool(name="xin", bufs=1) as xpool,
            tc.tile_pool(name="scratch", bufs=2) as spool,
            tc.tile_pool(name="small", bufs=1) as zpool,
        ):
            # x rows 4p+n for partition p: [128, 4*256]; each DMA half reads
            # 2KB contiguous per partition.  Issue on two queues in parallel.
            xt = xpool.tile([128, _NT * _F], f32)
            xr = x_d.ap().rearrange("(p n) f -> p (n f)", n=_NT)
            half = _NT * _F // 2
            nc.gpsimd.dma_start(xt[:, 0:half], xr[:, 0:half])
            wz = zpool.tile([128, _F + 2], f32)
            nc.sync.dma_start(wz[:], w_d[:, :])
            nc.sync.dma_start(xt[:, half:], xr[:, half:])

            # z'[p, n] = sum_f x[4p+n, f] * w'[f]
            z = zpool.tile([128, _NT], f32)
            for n in range(_NT):
                prod = spool.tile([128, _F], f32, name=f"prod{n}")
                nc.vector.scalar_tensor_tensor(
                    prod[:], xt[:, n * _F:(n + 1) * _F], 1.0, wz[:, 0:_F],
                    op0=Alu.mult, op1=Alu.mult,
                    accum_out=z[:, n:n + 1],
                )

            # range reduce: y = z' - round(z'), |y| <= 0.5
            kzi = zpool.tile([128, _NT], i32)
            nc.vector.tensor_copy(kzi[:], z[:])
            kzf = zpool.tile([128, _NT], f32)
            nc.vector.tensor_copy(kzf[:], kzi[:])
            y = zpool.tile([128, _NT], f32)
            nc.vector.tensor_tensor(y[:], z[:], kzf[:], op=Alu.subtract)
            # s = sin(pi*y); q = s^2; t = tanh(-c0*q + 0.5*c0) = tanh(a/2)
            s = zpool.tile([128, _NT], f32)
            nc.scalar.activation(s[:], y[:], Act.Sin, scale=_PI)
            q = zpool.tile([128, _NT], f32)
            nc.scalar.activation(q[:], s[:], Act.Square)
            t = zpool.tile([128, _NT], f32)
            nc.scalar.activation(t[:], q[:], Act.Tanh,
                                 scale=wz[:, _F:_F + 1],
                                 bias=wz[:, _F + 1:_F + 2])
            ot = zpool.tile([128, _NT], f32)
            nc.scalar.activation(ot[:], t[:], Act.Copy, scale=0.5, bias=0.5)

            nc.sync.dma_start(o_d.ap().rearrange("(p n) -> p n", n=_NT), ot[:])

    nc.compile()
    return nc


def _get_nc():
    if "nc" not in _CACHE:
        _CACHE["nc"] = _build()
    return _CACHE["nc"]


def _in_maps(x, W, params):
    x = np.ascontiguousarray(np.asarray(x, dtype=np.float32))
    W = np.asarray(W, dtype=np.float32)
    params = np.asarray(params, dtype=np.float32)
    wc = np.empty((128, _F + 2), dtype=np.float32)
    wc[:, :_F] = W[0] * _INV_TWO_PI
    wc[:, _F] = -math.cos(float(params[0]))
    wc[:, _F + 1] = 0.5 * math.cos(float(params[0]))
    return [
        {"x": x[c * _BS:(c + 1) * _BS], "w": wc}
        for c in range(_NCORES)
    ]


def run_spmd(x, W, params, **kw):
    """Compile (cached) and run on 8 cores; returns BassKernelResults.

    Retries a few times: the axon-relayed device occasionally reports a
    transient NRT_EXEC_UNIT_UNRECOVERABLE that clears on the next attempt.
    """
    import time

    from concourse import bass_utils

    nc = _get_nc()
    in_maps = _in_maps(x, W, params)
    last = None
    for attempt in range(4):
        try:
            return bass_utils.run_bass_kernel_spmd(
                nc, in_maps, list(range(_NCORES)), **kw
            )
        except Exception as e:  # transient device/relay errors
            last = e
            time.sleep(2.0 * (attempt + 1))
    raise last


def kernel(x, W, params):
    res = run_spmd(x, W, params)
    return np.concatenate([res.results[c]["o"] for c in range(_NCORES)], axis=0)


# revision 6
# speedup vs baseline: 1.1004x; 1.1004x over previous
"""Trainium2 Bass kernel for nn_GateCircuit (14-qubit batched gate circuit).

Math: the reference applies RX(x@W.T[:,i]) then RY(params[i]) on wire i of
|0...0> (a product state stays a product state since each gate hits a distinct
wire), then a CNOT ladder CNOT(i, i+1), then measures <Z_0>.  Qubit 0 is only
ever a CNOT *control*, so its marginal is untouched by the ladder; the
expectation collapses to the single-qubit value

    <Z_0> = cos(x @ W[0]) * cos(params[0])
    out   = sigmoid(<Z_0>)

Sharding: pure data parallel, batch 4096 split 512 per core across 8 cores.

Host prep (scalar/row transforms only; the 4MB matvec stays on-device):
  w' = W[0] / (2*pi), and a per-call least-squares fit of
  F(v) = sigmoid(c0*cos(2*pi*sqrt(v))) on v in [0, 0.25] as a degree-5
  polynomial (c0 = cos(params[0]) is known at call time; worst-case fit
  error 3.7e-4 abs vs the 2e-2 rel tolerance).  w' and the 6 coefficients
  pack into one [128, 262] tensor -> one DMA.

On-device per core (all f32, DVE only -- no ACT tables, no table loads):
  z' = x @ w'             4x DVE scalar_tensor_tensor with accumulator
                          (z' = z/2pi; x laid out [128, 4, 256]: partition p
                          holds rows 4p..4p+3, quarter-DMAs on the Sync +
                          Scalar hardware-DGE queues so tiles land in order)
  k  = f32(i32(z'))       round-to-nearest on HW cast
  y  = z' - k             in [-.5, .5];  cos(z) = cos(2*pi*y)
  v  = y*y                in [0, .25]
  out= F(v)               Horner via TS/STT alternation, coeffs as
                          per-partition [128,1] APs
"""

import math

import numpy as np

_NCORES = 8
_B = 4096
_F = 256
_BS = _B // _NCORES  # 512 samples per core
_NT = _BS // 128     # 4 samples per partition
_INV_TWO_PI = float(1.0 / (2.0 * math.pi))
_DEG = 5             # F(v) polynomial degree
_NC = _DEG + 1       # number of coefficients

_CACHE: dict = {}


def _build():
    import concourse.bacc as bacc
    import concourse.mybir as mybir
    import concourse.tile as tile

    f32 = mybir.dt.float32
    i32 = mybir.dt.int32
    Alu = mybir.AluOpType

    nc = bacc.Bacc("TRN2", target_bir_lowering=False, debug=False,
                   num_devices=_NCORES)

    x_d = nc.dram_tensor("x", [_BS, _F], f32, kind="ExternalInput")
    w_d = nc.dram_tensor("w", [128, _F + _NC], f32, kind="ExternalInput")
    o_d = nc.dram_tensor("o", [_BS], f32, kind="ExternalOutput")

    with tile.TileContext(nc) as tc:
        with (
            tc.tile_pool(name="xin", bufs=1) as xpool,
            tc.tile_pool(name="scratch", bufs=2) as spool,
            tc.tile_pool(name="small", bufs=1) as zpool,
        ):
            # x quarter n holds rows 4p+n (1KB contiguous per partition);
            # alternate the two free hardware-DGE queues (Scalar, Sync) so
            # quarters land roughly in order.  w' + coeffs go first on Sync.
            xt = xpool.tile([128, _NT * _F], f32)
            xr = x_d.ap().rearrange("(p n) f -> p (n f)", n=_NT)
            wz = zpool.tile([128, _F + _NC], f32)
            nc.scalar.dma_start(xt[:, 0:_F], xr[:, 0:_F])
            nc.sync.dma_start(wz[:], w_d[:, :])
            nc.sync.dma_start(xt[:, _F:2 * _F], xr[:, _F:2 * _F])
            nc.scalar.dma_start(xt[:, 2 * _F:3 * _F], xr[:, 2 * _F:3 * _F])
            nc.sync.dma_start(xt[:, 3 * _F:], xr[:, 3 * _F:])

            # z'[p, n] = sum_f x[4p+n, f] * w'[f]
            z = zpool.tile([128, _NT], f32)
            for n in range(_NT):
                prod = spool.tile([128, _F], f32, name=f"prod{n}")
                nc.vector.scalar_tensor_tensor(
                    prod[:], xt[:, n * _F:(n + 1) * _F], 1.0, wz[:, 0:_F],
                    op0=Alu.mult, op1=Alu.mult,
                    accum_out=z[:, n:n + 1],
                )

            # range reduce: y = z' - round(z'), |y| <= 0.5;  v = y^2
            kzi = zpool.tile([128, _NT], i32)
            nc.vector.tensor_copy(kzi[:], z[:])
            kzf = zpool.tile([128, _NT], f32)
            nc.vector.tensor_copy(kzf[:], kzi[:])
            y = zpool.tile([128, _NT], f32)
            nc.vector.tensor_tensor(y[:], z[:], kzf[:], op=Alu.subtract)
            v = zpool.tile([128, _NT], f32)
            nc.vector.tensor_tensor(v[:], y[:], y[:], op=Alu.mult)

            # out = f0 + v*(f1 + v*(f2 + v*(f3 + v*(f4 + v*f5))))
            # Horner with one op per coefficient: TS, bypass-mult, then
            # (add, mult) STT steps; final TS adds f0.
            def cf(k):
                return wz[:, _F + k:_F + k + 1]

            t1 = zpool.tile([128, _NT], f32)
            nc.vector.tensor_scalar(t1[:], v[:], cf(5), cf(4),
                                    op0=Alu.mult, op1=Alu.add)
            t2 = zpool.tile([128, _NT], f32)
            nc.vector.scalar_tensor_tensor(t2[:], t1[:], 0.0, v[:],
                                           op0=Alu.bypass, op1=Alu.mult)
            t3 = zpool.tile([128, _NT], f32)
            nc.vector.scalar_tensor_tensor(t3[:], t2[:], cf(3), v[:],
                                           op0=Alu.add, op1=Alu.mult)
            t4 = zpool.tile([128, _NT], f32)
            nc.vector.scalar_tensor_tensor(t4[:], t3[:], cf(2), v[:],
                                           op0=Alu.add, op1=Alu.mult)
            t5 = zpool.tile([128, _NT], f32)
            nc.vector.scalar_tensor_tensor(t5[:], t4[:], cf(1), v[:],
                                           op0=Alu.add, op1=Alu.mult)
            ot = zpool.tile([128, _NT], f32)
            nc.vector.tensor_scalar(ot[:], t5[:], 1.0, cf(0),
                                    op0=Alu.mult, op1=Alu.add)

            nc.sync.dma_start(o_d.ap().rearrange("(p n) -> p n", n=_NT), ot[:])

    nc.compile()
    return nc


def _get_nc():
    if "nc" not in _CACHE:
        _CACHE["nc"] = _build()
    return _CACHE["nc"]


def _fit_coeffs(c0: float) -> np.ndarray:
    """Least-squares fit of sigmoid(c0*cos(2*pi*sqrt(v))) on v in [0,.25],
    degree _DEG, on Chebyshev-spaced nodes (near-minimax)."""
    t = 0.5 * (1.0 - np.cos(np.pi * np.linspace(0.0, 1.0, 401))) * 0.25
    F = 1.0 / (1.0 + np.exp(-c0 * np.cos(2.0 * np.pi * np.sqrt(t))))
    A = np.stack([t ** k for k in range(_NC)], axis=1)
    coef, *_ = np.linalg.lstsq(A, F, rcond=None)
    return coef.astype(np.float32)


def _in_maps(x, W, params):
    x = np.ascontiguousarray(np.asarray(x, dtype=np.float32))
    W = np.asarray(W, dtype=np.float32)
    params = np.asarray(params, dtype=np.float32)
    wc = np.empty((128, _F + _NC), dtype=np.float32)
    wc[:, :_F] = W[0] * _INV_TWO_PI
    wc[:, _F:] = _fit_coeffs(math.cos(float(params[0])))
    return [
        {"x": x[c * _BS:(c + 1) * _BS], "w": wc}
        for c in range(_NCORES)
    ]


def run_spmd(x, W, params, **kw):
    """Compile (cached) and run on 8 cores; returns BassKernelResults.

    Retries a few times: the axon-relayed device occasionally reports a
    transient NRT_EXEC_UNIT_UNRECOVERABLE that clears on the next attempt.
    """
    import time

    from concourse import bass_utils

    nc = _get_nc()
    in_maps = _in_maps(x, W, params)
    last = None
    for attempt in range(4):
        try:
            return bass_utils.run_bass_kernel_spmd(
                nc, in_maps, list(range(_NCORES)), **kw
            )
        except Exception as e:  # transient device/relay errors
            last = e
            time.sleep(2.0 * (attempt + 1))
    raise last


def kernel(x, W, params):
    res = run_spmd(x, W, params)
    return np.concatenate([res.results[c]["o"] for c in range(_NCORES)], axis=0)
